# revision 3
# baseline (speedup 1.0000x reference)
"""Bass/Trainium2 kernel for the decomposed LocallyConnected2d layer.

out[b,o,i,j] = sum_{c,k} x[b, c, i+di, j+dj] * w[o, c, i, j, k] + bias[o,i,j]
with k = di*3 + dj (3x3 kernel, stride 1).

Strategy: shard over output rows i across 8 cores (4 rows each). Per output
location (i,j) the contraction (c,di,dj)=288 is split into 3 chunks of 96 =
(di,c), chunked over dj; each chunk is one matmul lhsT=[96,64] rhs=[96,128]
accumulating into PSUM [64 o, 128 b]. The bias is folded into the dj=2 chunk
as a 97th contraction row against a constant-ones rhs partition. Even/odd j
use PE column groups 0/1 (tile_position) so two locations' matmuls overlap.

Dtypes: the kernel is HBM-byte-bound, so weight traffic is cut with fp8.
Chunks dj=0,1 ship as fp8-e4m3 (scaled x256 to avoid the subnormal range);
chunk dj=2 + bias stay fp16 so the quantization error keeps margin under the
2e-2 gate (measured 1.36e-2 on the real seed). The PSUM->SBUF copies apply
the 1/256 compensation. rhs (x) and output are fp16.

DMA design: per-core traffic is ~8.6 MB against a ~270 GB/s effective
per-core DMA ceiling, so the goal is a dense two-queue stream with
progressive gating. x ships as four per-output-row slab tensors (partition
p = di*32+c holds image row i+di; partition 96 = ones) on the scalar HWDGE
queue; weights as six row-half chunk tiles on the sync queue in consumption
order; output leaves as two 2-row DMAs appended to the sync ring so they
drain after the weights. 12 DMAs total, every run >= 4KB.
"""

import sys

for _p in ("/opt/trn_rl_repo", "/root/.axon_site/_ro/trn_rl_repo"):
    if _p not in sys.path:
        sys.path.append(_p)

import numpy as np

B = 128
C_IN = 32
C_OUT = 64
OH = OW = 32
KH = KW = 3
H = W = 34
N_CORES = 8
RPC = OH // N_CORES          # output rows per core = 4
HALO = RPC + KH - 1          # x rows per core = 6
NPAIR = OW // 2              # j-pairs per row = 16
NGRP = 4                     # j-pairs per psum group
GRPS = NPAIR // NGRP         # psum groups per row = 4
WSCALE = 256.0               # weight scale to keep fp8 out of subnormals

_prog_cache = {}


def _build_program():
    import concourse.tile as tile
    from concourse import bacc, mybir

    f16 = mybir.dt.float16
    f8 = mybir.dt.float8e4
    f32 = mybir.dt.float32

    nc = bacc.Bacc("TRN2", target_bir_lowering=False, debug=False,
                   num_devices=N_CORES)

    # Per-core DRAM I/O (host pre-sharded / pre-transposed):
    #   xs  [97, i=4, w=34, b=128] f16   partition p=di*32+c holds x row
    #                                    r0+i+di; p=96 is all-ones
    #   wa/wb [96, i=4, j=32, o=64] f8   dj=0/1 chunks, scaled x256
    #   wc  [97, i=4, j=32, o=64] f16    dj=2 chunk + bias row, scaled x256
    #   out [p2=128 (par*64+o), i=4, jh=16, b=128] f16 ; j = 2*jh + par
    xs_in = nc.dram_tensor("xs", [97, RPC, W, B], f16,
                           kind="ExternalInput").ap()
    wa_in = nc.dram_tensor("wa", [96, RPC, OW, C_OUT], f8,
                           kind="ExternalInput").ap()
    wb_in = nc.dram_tensor("wb", [96, RPC, OW, C_OUT], f8,
                           kind="ExternalInput").ap()
    wc_in = nc.dram_tensor("wc", [97, RPC, OW, C_OUT], f16,
                           kind="ExternalInput").ap()
    out = nc.dram_tensor("out", [128, RPC, NPAIR, B], f16,
                         kind="ExternalOutput").ap()

    with tile.TileContext(nc) as tc:
        with (
            tc.tile_pool(name="xpool", bufs=1) as xpool,
            tc.tile_pool(name="wpool", bufs=1) as wpool,
            tc.tile_pool(name="opool", bufs=2) as opool,
            tc.tile_pool(name="pspool", bufs=6, space="PSUM") as pspool,
        ):
            # x: one tile per output row so row i's matmuls gate only on its
            # own 0.85MB slab. Issued first on the scalar HWDGE queue.
            xr = [xpool.tile([97, W, B], f16, tag=f"x{i}", name=f"x{i}")
                  for i in range(RPC)]
            for i in range(RPC):
                nc.scalar.dma_start(xr[i][:], xs_in[:, i])

            # w: row-half tiles so rows 0-1 don't wait for rows 2-3 weights.
            # Sync queue, consumption order; the ring drains FIFO so this is
            # also the transfer priority.
            wa0 = wpool.tile([96, 2, OW, C_OUT], f8, tag="wa0")
            wb0 = wpool.tile([96, 2, OW, C_OUT], f8, tag="wb0")
            wc0 = wpool.tile([97, 2, OW, C_OUT], f16, tag="wc0")
            wa1 = wpool.tile([96, 2, OW, C_OUT], f8, tag="wa1")
            wb1 = wpool.tile([96, 2, OW, C_OUT], f8, tag="wb1")
            wc1 = wpool.tile([97, 2, OW, C_OUT], f16, tag="wc1")
            for h, (ta, tb, tcw) in enumerate([(wa0, wb0, wc0),
                                               (wa1, wb1, wc1)]):
                rs = slice(2 * h, 2 * h + 2)
                nc.sync.dma_start(ta[:], wa_in[:, rs])
                nc.sync.dma_start(tb[:], wb_in[:, rs])
                nc.sync.dma_start(tcw[:], wc_in[:, rs])
            wa_h, wb_h, wc_h = [wa0, wa1], [wb0, wb1], [wc0, wc1]

            inv_s = 1.0 / WSCALE
            orow = None
            for i in range(RPC):
                if i % 2 == 0:
                    orow = opool.tile([128, 2, NPAIR, B], f16, tag=f"o{i//2}")
                wa, wb, wc = wa_h[i // 2], wb_h[i // 2], wc_h[i // 2]
                ii = i % 2
                xt = xr[i]
                for g in range(GRPS):
                    ps = pspool.tile([128, NGRP, B], f32)
                    for pig in range(NGRP):
                        for par in range(2):
                            j = 2 * (NGRP * g + pig) + par
                            pslice = ps[64 * par:64 * par + 64, pig, :]
                            tp = (0, 64 * par)
                            nc.tensor.matmul(pslice, wa[:, ii, j, :],
                                             xt[0:96, j, :],
                                             start=True, stop=False,
                                             tile_position=tp)
                            nc.tensor.matmul(pslice, wb[:, ii, j, :],
                                             xt[0:96, j + 1, :],
                                             start=False, stop=False,
                                             tile_position=tp)
                            nc.tensor.matmul(pslice, wc[:, ii, j, :],
                                             xt[0:97, j + 2, :],
                                             start=False, stop=True,
                                             tile_position=tp)
                    dst = orow[:, ii, NGRP * g:NGRP * (g + 1), :]
                    if g % 2 == 0:
                        nc.vector.tensor_scalar_mul(dst, ps[:], inv_s)
                    else:
                        nc.scalar.mul(dst, ps[:], inv_s)
                if i % 2 == 1:
                    nc.sync.dma_start(out[:, i - 1:i + 1], orow[:])

    nc.compile()
    return nc


def _host_prep(x, weight, bias):
    """Full fp32 inputs -> list of per-core input dicts."""
    import ml_dtypes
    f8 = ml_dtypes.float8_e4m3

    # x: (B, C, H, W) -> (C, H, W, B) fp16
    x_t = np.ascontiguousarray(x.transpose(1, 2, 3, 0)).astype(np.float16)
    # w: (O, C, I, J, K) with K=(di*3+dj) -> per dj: [(di*32+c)=96, I, J, O]
    w_r = (weight * WSCALE).reshape(C_OUT, C_IN, OH, OW, KH, KW)
    w_t = w_r.transpose(5, 4, 1, 2, 3, 0)          # (dj, di, c, I, J, O)
    w_t = w_t.reshape(KW, 96, OH, OW, C_OUT)
    wa_full = w_t[0].astype(f8)
    wb_full = w_t[1].astype(f8)
    b_t = (bias * WSCALE).transpose(1, 2, 0)[None]  # (1, I, J, O)
    wc_full = np.concatenate([w_t[2], b_t], axis=0).astype(np.float16)

    in_maps = []
    for m in range(N_CORES):
        r0 = m * RPC
        # x slab: [97, RPC, W, B]; partition di*32+c, row index i -> r0+i+di
        xs = np.empty((97, RPC, W, B), np.float16)
        xsv = xs[:96].reshape(KH, C_IN, RPC, W, B)
        for di in range(KH):
            # rows r0+di .. r0+di+RPC-1, transposed to (C, RPC, W, B)
            xsv[di] = x_t[:, r0 + di:r0 + di + RPC]
        xs[96] = 1.0
        in_maps.append({
            "xs": xs,
            "wa": np.ascontiguousarray(wa_full[:, r0:r0 + RPC]),
            "wb": np.ascontiguousarray(wb_full[:, r0:r0 + RPC]),
            "wc": np.ascontiguousarray(wc_full[:, r0:r0 + RPC]),
        })
    return in_maps


def _gather(results):
    out_full = np.empty((B, C_OUT, OH, OW), np.float32)
    for m in range(N_CORES):
        r = results[m]["out"].astype(np.float32)          # (128, 4, 16, 128)
        r = r.reshape(2, C_OUT, RPC, NPAIR, B)            # par,o,i,jh,b
        r = r.transpose(4, 1, 2, 3, 0)                    # b,o,i,jh,par
        out_full[:, :, m * RPC:(m + 1) * RPC, :] = r.reshape(B, C_OUT, RPC, OW)
    return out_full


def kernel(x, weight, bias, _trace=False):
    from concourse.bass_utils import run_bass_kernel_spmd

    if "nc" not in _prog_cache:
        _prog_cache["nc"] = _build_program()
    nc = _prog_cache["nc"]

    in_maps = _host_prep(np.asarray(x), np.asarray(weight), np.asarray(bias))
    res = run_bass_kernel_spmd(nc, in_maps, core_ids=list(range(N_CORES)),
                               trace=_trace)
    out = _gather(res.results)
    if _trace:
        _prog_cache["last_result"] = res
    return out


# revision 8
# speedup vs baseline: 4.0113x; 4.0113x over previous
"""Bass/Trainium2 kernel for the decomposed LocallyConnected2d layer.

out[b,o,i,j] = sum_{c,k} x[b, c, i+di, j+dj] * w[o, c, i, j, k] + bias[o,i,j]
with k = di*3 + dj (3x3 kernel, stride 1).

Strategy: shard over output rows i across 8 cores (4 rows each). Per output
location (i,j) the contraction (c,di,dj)=288 is split into 3 chunks of 96 =
(di,c), chunked over dj; each chunk is one matmul lhsT=[96,64] rhs=[96,128]
accumulating into PSUM [64 o, 128 b]. The bias is folded into the dj=2 chunk
as a 97th contraction row against a constant-ones rhs partition. Even/odd j
use PE column groups 0/1 (tile_position) so two locations' matmuls overlap.

Dtypes: the kernel is HBM-byte-bound, so weight traffic is cut with fp8.
Chunks dj=0,1 ship as fp8-e4m3 (scaled x256 to avoid the subnormal range);
chunk dj=2 + bias stay fp16 so the quantization error keeps margin under the
2e-2 gate (measured 1.36e-2 on the real seed). The PSUM->SBUF copies apply
the 1/256 compensation. rhs (x) and output are fp16.

DMA design: per-core traffic is ~8.6 MB against a ~270 GB/s effective
per-core DMA ceiling, so the goal is a dense two-queue stream with
progressive gating. x ships as two row-pair slab tensors (partition
p = di*32+c holds image row i+di; partition 96 = ones) on the scalar HWDGE
queue; weights as six row-half chunk tiles on the sync queue in consumption
order; output leaves as two 2-row DMAs on the scalar ring once it drains.
The HWDGE only spreads a DMA across the 16 engines when the outermost AP
dim is a multiple of 16, so bulk DMAs are 96/128 partitions and the ones /
bias rows (partition 96) move as tiny single-packet DMAs issued first.
"""

import sys

for _p in ("/opt/trn_rl_repo", "/root/.axon_site/_ro/trn_rl_repo"):
    if _p not in sys.path:
        sys.path.append(_p)

import numpy as np

B = 128
C_IN = 32
C_OUT = 64
OH = OW = 32
KH = KW = 3
H = W = 34
N_CORES = 8
RPC = OH // N_CORES          # output rows per core = 4
HALO = RPC + KH - 1          # x rows per core = 6
NPAIR = OW // 2              # j-pairs per row = 16
NGRP = 4                     # j-pairs per psum group
GRPS = NPAIR // NGRP         # psum groups per row = 4
WSCALE = 256.0               # weight scale to keep fp8 out of subnormals

_prog_cache = {}


def _build_program():
    import concourse.tile as tile
    from concourse import bacc, mybir

    f16 = mybir.dt.float16
    f8 = mybir.dt.float8e4
    f32 = mybir.dt.float32

    nc = bacc.Bacc("TRN2", target_bir_lowering=False, debug=False,
                   num_devices=N_CORES)

    # Per-core DRAM I/O (host pre-sharded / pre-transposed):
    #   xs  [97, i=4, w=34, b=128] f16   partition p=di*32+c holds x row
    #                                    r0+i+di; p=96 is all-ones
    #   wa/wb [96, i=4, j=32, o=64] f8   dj=0/1 chunks, scaled x256
    #   wc  [97, i=4, j=32, o=64] f16    dj=2 chunk + bias row, scaled x256
    #   out [p2=128 (par*64+o), i=4, jh=16, b=128] f16 ; j = 2*jh + par
    xs_in = nc.dram_tensor("xs", [97, RPC, W, B], f16,
                           kind="ExternalInput").ap()
    wa_in = nc.dram_tensor("wa", [96, RPC, OW, C_OUT], f8,
                           kind="ExternalInput").ap()
    wb_in = nc.dram_tensor("wb", [96, RPC, OW, C_OUT], f8,
                           kind="ExternalInput").ap()
    wc_in = nc.dram_tensor("wc", [97, RPC, OW, C_OUT], f16,
                           kind="ExternalInput").ap()
    out = nc.dram_tensor("out", [128, RPC, NPAIR, B], f16,
                         kind="ExternalOutput").ap()

    with tile.TileContext(nc) as tc:
        with (
            tc.tile_pool(name="xpool", bufs=1) as xpool,
            tc.tile_pool(name="wpool", bufs=1) as wpool,
            tc.tile_pool(name="opool", bufs=2) as opool,
            tc.tile_pool(name="pspool", bufs=6, space="PSUM") as pspool,
        ):
            # x: one tile per output-row PAIR so rows 01 gate only on their
            # half of the slab. The HWDGE spreads a DMA's packets across the
            # 16 DMA engines by its outermost AP dim, and only multiples of
            # 16 spread (97 degenerates to a single engine) — so every bulk
            # DMA below is 96 or 128 partitions, and the 97th (ones / bias)
            # rows move as separate tiny single-packet DMAs issued first.
            xA = xpool.tile([97, 2, W, B], f16, tag="xA")
            xB = xpool.tile([97, 2, W, B], f16, tag="xB")
            wa0 = wpool.tile([96, 2, OW, C_OUT], f8, tag="wa0")
            wb0 = wpool.tile([96, 2, OW, C_OUT], f8, tag="wb0")
            wc0 = wpool.tile([97, 2, OW, C_OUT], f16, tag="wc0")
            wa1 = wpool.tile([96, 2, OW, C_OUT], f8, tag="wa1")
            wb1 = wpool.tile([96, 2, OW, C_OUT], f8, tag="wb1")
            wc1 = wpool.tile([97, 2, OW, C_OUT], f16, tag="wc1")

            # tiny 1-partition DMAs first: they complete immediately, so
            # when the ~10-deep DMA-completion-sem pool recycles their sems
            # for the late bulk DMAs, nothing chains behind a live transfer.
            nc.scalar.dma_start(xA[96:97, :, :, :], xs_in[96:97, 0:2])
            nc.scalar.dma_start(xB[96:97, :, :, :], xs_in[96:97, 2:4])
            nc.sync.dma_start(wc0[96:97, :, :, :], wc_in[96:97, 0:2])
            nc.sync.dma_start(wc1[96:97, :, :, :], wc_in[96:97, 2:4])

            # bulk: x pair-slabs on the scalar queue (34.8KB runs), weights
            # on the sync queue in consumption order (ring drains FIFO, so
            # issue order is transfer priority).
            nc.scalar.dma_start(xA[0:96, :, :, :], xs_in[0:96, 0:2])
            nc.scalar.dma_start(xB[0:96, :, :, :], xs_in[0:96, 2:4])
            for h, (ta, tb, tcw) in enumerate([(wa0, wb0, wc0),
                                               (wa1, wb1, wc1)]):
                rs = slice(2 * h, 2 * h + 2)
                nc.sync.dma_start(ta[:], wa_in[:, rs])
                nc.sync.dma_start(tb[:], wb_in[:, rs])
                nc.sync.dma_start(tcw[0:96, :, :, :], wc_in[0:96, rs])
            wa_h, wb_h, wc_h = [wa0, wa1], [wb0, wb1], [wc0, wc1]
            x_h = [xA, xB]

            inv_s = 1.0 / WSCALE
            orow = None
            for i in range(RPC):
                if i % 2 == 0:
                    orow = opool.tile([128, 2, NPAIR, B], f16, tag=f"o{i//2}")
                wa, wb, wc = wa_h[i // 2], wb_h[i // 2], wc_h[i // 2]
                ii = i % 2
                xt = x_h[i // 2]
                for g in range(GRPS):
                    ps = pspool.tile([128, NGRP, B], f32)
                    for pig in range(NGRP):
                        for par in range(2):
                            j = 2 * (NGRP * g + pig) + par
                            pslice = ps[64 * par:64 * par + 64, pig, :]
                            tp = (0, 64 * par)
                            nc.tensor.matmul(pslice, wa[:, ii, j, :],
                                             xt[0:96, ii, j, :],
                                             start=True, stop=False,
                                             tile_position=tp)
                            nc.tensor.matmul(pslice, wb[:, ii, j, :],
                                             xt[0:96, ii, j + 1, :],
                                             start=False, stop=False,
                                             tile_position=tp)
                            nc.tensor.matmul(pslice, wc[:, ii, j, :],
                                             xt[0:97, ii, j + 2, :],
                                             start=False, stop=True,
                                             tile_position=tp)
                    dst = orow[:, ii, NGRP * g:NGRP * (g + 1), :]
                    if g % 2 == 0:
                        nc.vector.tensor_scalar_mul(dst, ps[:], inv_s)
                    else:
                        nc.scalar.mul(dst, ps[:], inv_s)
                if i % 2 == 1:
                    nc.scalar.dma_start(out[:, i - 1:i + 1], orow[:])

    nc.compile()
    return nc


def _host_prep(x, weight, bias):
    """Full fp32 inputs -> list of per-core input dicts."""
    import ml_dtypes
    f8 = ml_dtypes.float8_e4m3

    # x: (B, C, H, W) -> (C, H, W, B) fp16
    x_t = np.ascontiguousarray(x.transpose(1, 2, 3, 0)).astype(np.float16)
    # w: (O, C, I, J, K) with K=(di*3+dj) -> per dj: [(di*32+c)=96, I, J, O]
    w_r = (weight * WSCALE).reshape(C_OUT, C_IN, OH, OW, KH, KW)
    w_t = w_r.transpose(5, 4, 1, 2, 3, 0)          # (dj, di, c, I, J, O)
    w_t = w_t.reshape(KW, 96, OH, OW, C_OUT)
    wa_full = w_t[0].astype(f8)
    wb_full = w_t[1].astype(f8)
    b_t = (bias * WSCALE).transpose(1, 2, 0)[None]  # (1, I, J, O)
    wc_full = np.concatenate([w_t[2], b_t], axis=0).astype(np.float16)

    in_maps = []
    for m in range(N_CORES):
        r0 = m * RPC
        # x slab: [97, RPC, W, B]; partition di*32+c, row index i -> r0+i+di
        xs = np.empty((97, RPC, W, B), np.float16)
        xsv = xs[:96].reshape(KH, C_IN, RPC, W, B)
        for di in range(KH):
            # rows r0+di .. r0+di+RPC-1, transposed to (C, RPC, W, B)
            xsv[di] = x_t[:, r0 + di:r0 + di + RPC]
        xs[96] = 1.0
        in_maps.append({
            "xs": xs,
            "wa": np.ascontiguousarray(wa_full[:, r0:r0 + RPC]),
            "wb": np.ascontiguousarray(wb_full[:, r0:r0 + RPC]),
            "wc": np.ascontiguousarray(wc_full[:, r0:r0 + RPC]),
        })
    return in_maps


def _gather(results):
    out_full = np.empty((B, C_OUT, OH, OW), np.float32)
    for m in range(N_CORES):
        r = results[m]["out"].astype(np.float32)          # (128, 4, 16, 128)
        r = r.reshape(2, C_OUT, RPC, NPAIR, B)            # par,o,i,jh,b
        r = r.transpose(4, 1, 2, 3, 0)                    # b,o,i,jh,par
        out_full[:, :, m * RPC:(m + 1) * RPC, :] = r.reshape(B, C_OUT, RPC, OW)
    return out_full


def kernel(x, weight, bias, _trace=False):
    from concourse.bass_utils import run_bass_kernel_spmd

    if "nc" not in _prog_cache:
        _prog_cache["nc"] = _build_program()
    nc = _prog_cache["nc"]

    in_maps = _host_prep(np.asarray(x), np.asarray(weight), np.asarray(bias))
    res = run_bass_kernel_spmd(nc, in_maps, core_ids=list(range(N_CORES)),
                               trace=_trace)
    out = _gather(res.results)
    if _trace:
        _prog_cache["last_result"] = res
    return out
